# revision 14
# baseline (speedup 1.0000x reference)
"""Distributed Trainium2 Bass kernel for masked multi-head attention.

Problem: out = (softmax(scale * x Wq^T (x Wk^T)^T + mask * -1e5) (x Wv^T)) Wp^T + bp
  x [4, 2048, 768], mask [4, 2048, 2048], H=12 heads, D=64.

Sharding (8 cores): core = (batch b, head-group hg) with b = core//2,
hg = core%2 (6 heads each).  Column-parallel Wq/Wkv, row-parallel Wp;
each core produces a partial [2048, 768] output; the host sums the two
head-group partials per batch and adds the bias, then stacks batches.

Device schedule: the ACT engine's exp stream (192 tiles x ~1.03us) is
the pace setter; everything is arranged so it starts early and rarely
waits:
  - Minimal prefix (Q/K projection of head-pair 0 / block 0 on
    fine-grained x^T DMA) so the first QK->exp fires at ~8us instead of
    after a 60us serial projection phase.
  - All other projections drip into the attention iteration stream as
    PE filler.  Their PSUM comes from the shared "ot" slots, kept free
    by deferring early PV matmuls (pm tiles queue in a deep SBUF pool
    and drain after the per-qchunk O accumulators allocate); overflow
    units borrow "ring" slots once the otiles exist.
  - qchunk boundaries: previous epilogue (softmax divide / transpose /
    out-projection) and next q-block projections run in a deferred-PV
    window at the top of each qchunk; divisions are emitted first in
    DVE program order so the otile slots free immediately.
  - Engines: ACT = exp only; DVE = mask-mul + divisions + even-head
    evictions; Pool = odd-head evictions, psum->sbuf copies, mask DMA
    issue; SP = x/w/out DMA.
  - qt/kt pack head PAIRS on partitions (rows 0:64 even head, 64:128
    odd head): QK contracts over 64 rows directly, evictions stay on
    their own partitions, SBUF halves.
"""

import os
from collections import deque
from contextlib import ExitStack

import ml_dtypes
import numpy as np

import sys
import types

try:  # defensive: concourse's trace path imports this; absent on some images
    import antenv.axon_hooks  # noqa: F401
except ImportError:
    try:
        import antenv
        _m = types.ModuleType('antenv.axon_hooks')
        _m._hook = None
        _m.set_axon_ntff_profile_hook = lambda h: setattr(_m, '_hook', h)
        _m.get_axon_ntff_profile_hook = lambda: _m._hook
        sys.modules['antenv.axon_hooks'] = _m
        antenv.axon_hooks = _m
    except ImportError:
        pass

import concourse.bass as bass
import concourse.tile as tile
from concourse import bacc, mybir
from concourse.bass_utils import run_bass_kernel_spmd
from concourse.masks import make_identity

B, N, C, H, D = 4, 2048, 768, 12, 64
SCALE = D ** -0.5
NCORES = 8
HGROUPS = 2
HL = H // HGROUPS          # 6 heads per group
CH = HL * D                # 384 channels per group
P = 128
NKT = N // P               # 16 k tiles
QCHUNK = 512
NQC = N // QCHUNK          # 4 q chunks
QSUBS = QCHUNK // P        # 4
CIN_T = C // P             # 6 input-channel tiles
CH_T = CH // P             # 3 group-channel tiles
MP = HL // 2               # 3 head pairs per group
E = D + 1                  # head slot width in O psum (64 V cols + 1 ones col)
NIT = HL * NKT // 2        # 48 iterations per qchunk

F32 = mybir.dt.float32
BF16 = mybir.dt.bfloat16


def build_kernel():
    nc = bacc.Bacc("TRN2", target_bir_lowering=False, debug=False,
                   num_devices=NCORES)

    xT = nc.dram_tensor("xT", [C, N], BF16, kind="ExternalInput").ap()
    wqt = nc.dram_tensor("wqt", [C, CH], BF16, kind="ExternalInput").ap()
    wkt = nc.dram_tensor("wkt", [C, CH], BF16, kind="ExternalInput").ap()
    wvt = nc.dram_tensor("wvt", [C, CH], BF16, kind="ExternalInput").ap()
    wpt = nc.dram_tensor("wpt", [CH, C], BF16, kind="ExternalInput").ap()
    negmt = nc.dram_tensor("negmt", [N, N], BF16, kind="ExternalInput").ap()
    out = nc.dram_tensor("out", [N, C], F32, kind="ExternalOutput").ap()

    with tile.TileContext(nc) as tc, ExitStack() as ctx:
        persist = ctx.enter_context(tc.tile_pool(name="persist", bufs=1))
        ring_pool = ctx.enter_context(
            tc.tile_pool(name="ring", bufs=2, space="PSUM"))
        o_pool = ctx.enter_context(
            tc.tile_pool(name="opsum", bufs=4, space="PSUM"))

        # head-pair packed Q^T / K^T: rows 0:64 head 2m, rows 64:128 head 2m+1
        qt_sb = [persist.tile([P, N], BF16, tag=f"qt{m}", name=f"qt{m}")
                 for m in range(MP)]
        kt_sb = [persist.tile([P, N], BF16, tag=f"kt{m}", name=f"kt{m}")
                 for m in range(MP)]
        vp_sb = [persist.tile([P, HL, E], BF16, tag=f"vp{j}", name=f"vp{j}")
                 for j in range(NKT)]
        wp_sb = [persist.tile([P, C], BF16, tag=f"wp{t}", name=f"wp{t}")
                 for t in range(CH_T)]
        idn = persist.tile([P, P], BF16, tag="idn")

        ph1 = ctx.enter_context(tc.tile_pool(name="ph1", bufs=1))
        xt_sb = [ph1.tile([P, N], BF16, tag=f"xt{i}", name=f"xt{i}")
                 for i in range(CIN_T)]
        wq_sb = [ph1.tile([P, CH], BF16, tag=f"wq{i}", name=f"wq{i}")
                 for i in range(CIN_T)]
        wk_sb = [ph1.tile([P, CH], BF16, tag=f"wk{i}", name=f"wk{i}")
                 for i in range(CIN_T)]
        wv_sb = [ph1.tile([P, CH], BF16, tag=f"wv{i}", name=f"wv{i}")
                 for i in range(CIN_T)]

        # ---- DMA issue (SP queue): Q/K weights first, then the x^T chunk
        # feeding q-block 0 / k-tiles 0..3, then the rest.
        for i in range(CIN_T):
            sl = slice(i * P, (i + 1) * P)
            nc.sync.dma_start(out=wq_sb[i], in_=wqt[sl, :])
            nc.sync.dma_start(out=wk_sb[i], in_=wkt[sl, :])
        for i in range(CIN_T):
            nc.sync.dma_start(out=xt_sb[i][:, 0:QCHUNK],
                              in_=xT[i * P:(i + 1) * P, 0:QCHUNK])
        for i in range(CIN_T):
            sl = slice(i * P, (i + 1) * P)
            nc.sync.dma_start(out=wv_sb[i], in_=wvt[sl, :])
        for nck in range(1, NQC):
            q0 = nck * QCHUNK
            for i in range(CIN_T):
                nc.sync.dma_start(out=xt_sb[i][:, q0:q0 + QCHUNK],
                                  in_=xT[i * P:(i + 1) * P, q0:q0 + QCHUNK])
        for t in range(CH_T):
            nc.sync.dma_start(out=wp_sb[t], in_=wpt[t * P:(t + 1) * P, :])

        # ---- mask tiles (Pool queue; single slot, issued at qc end) ----
        mpool = ctx.enter_context(tc.tile_pool(name="mask", bufs=1))
        mk_tiles = {}

        def issue_mask(qc):
            q0 = qc * QCHUNK
            mk = mpool.tile([P, NKT, QCHUNK], BF16, tag="mk", name=f"mk{qc}")
            for j in range(NKT):
                nc.gpsimd.dma_start(
                    out=mk[:, j, :],
                    in_=negmt[j * P:(j + 1) * P, q0:q0 + QCHUNK])
            mk_tiles[qc] = mk

        issue_mask(0)

        # ---- pointwise pools ----
        p_pool = ctx.enter_context(tc.tile_pool(name="pexp", bufs=6))
        pm_pool = ctx.enter_context(tc.tile_pool(name="pmask", bufs=28))
        epi = ctx.enter_context(tc.tile_pool(name="epi", bufs=6))
        otsb_pool = ctx.enter_context(tc.tile_pool(name="otsb", bufs=4))
        outsb_pool = ctx.enter_context(tc.tile_pool(name="outsb", bufs=3))

        # ---- projection units (one psum slot + eviction each) ----
        v_emitted = set()

        def unit_q(m, n, w_sb, dst):
            ps = o_pool.tile([P, QCHUNK], F32, tag="ot", name=f"pj{m}_{n}")
            for ci in range(CIN_T):
                nc.tensor.matmul(
                    ps,
                    w_sb[ci][:, m * P:(m + 1) * P],
                    xt_sb[ci][:, n * QCHUNK:(n + 1) * QCHUNK],
                    start=(ci == 0), stop=(ci == CIN_T - 1))
            dt = dst[m]
            nsl = slice(n * QCHUNK, (n + 1) * QCHUNK)
            # both packed heads in one copy (GPSIMD cannot read PSUM)
            nc.vector.tensor_copy(dt[:, nsl], ps)

        def unit_v(j):
            ps = o_pool.tile([P, CH], F32, tag="ot", name=f"vps{j}")
            for ci in range(CIN_T):
                nc.tensor.matmul(
                    ps,
                    xt_sb[ci][:, j * P:(j + 1) * P],
                    wv_sb[ci],
                    start=(ci == 0), stop=(ci == CIN_T - 1))
            nc.gpsimd.memset(vp_sb[j], 1.0)
            nc.vector.tensor_copy(
                vp_sb[j][:, :, 0:D],
                ps.rearrange("p (h d) -> p h d", h=HL))
            v_emitted.add(j)

        # ---- attention iteration pieces ----
        pvq = deque()          # deferred PV groups: (h, ktp, pm, otiles)
        mask_ctr = [0]

        def emit_qk_exp_mask(qc, h, ktp, otiles):
            m, r = divmod(h, 2)
            rows = slice(r * D, (r + 1) * D)
            q0 = qc * QCHUNK
            ring = ring_pool.tile([P, 2, QCHUNK], F32, tag="ring",
                                  name=f"ring{qc}_{h}_{ktp}")
            for u in range(2):
                kti = 2 * ktp + u
                nc.tensor.matmul(
                    ring[:, u, :],
                    kt_sb[m][rows, kti * P:(kti + 1) * P],
                    qt_sb[m][rows, q0:q0 + QCHUNK],
                    start=True, stop=True)
            pexp = p_pool.tile([P, 2, QCHUNK], BF16, tag="pe",
                               name=f"pe{qc}_{h}_{ktp}")
            nc.scalar.activation(
                pexp, ring, mybir.ActivationFunctionType.Exp)
            pm = pm_pool.tile([P, 2, QCHUNK], BF16, tag="pm",
                              name=f"pm{qc}_{h}_{ktp}")
            # every 4th mask-multiply on Pool (all-SBUF, so it's legal
            # there) to keep DVE below the ACT exp pace
            mask_ctr[0] += 1
            eng = nc.gpsimd if mask_ctr[0] % 4 == 0 else nc.vector
            eng.tensor_mul(
                pm, pexp, mk_tiles[qc][:, 2 * ktp:2 * ktp + 2, :])
            pvq.append((h, ktp, pm, otiles))

        def front_ready(qc):
            if qc > 0:
                return True
            ktp = pvq[0][1]
            return 2 * ktp in v_emitted and 2 * ktp + 1 in v_emitted

        def emit_pv_group():
            h, ktp, pm, otiles = pvq.popleft()
            for u in range(2):
                kti = 2 * ktp + u
                for s in range(QSUBS):
                    nc.tensor.matmul(
                        otiles[s][:, h * E:(h + 1) * E],
                        pm[:, u, s * P:(s + 1) * P],
                        vp_sb[kti][:, h, :],
                        start=(kti == 0), stop=(kti == NKT - 1))

        # ---- epilogue: divisions first, then per-s-tile chunks ----
        def epi_divisions(qc, otiles):
            osbs = []
            for s in range(QSUBS):
                otv = otiles[s].rearrange("p (h e) -> p h e", h=HL)
                zrec = epi.tile([P, HL], F32, tag="zr", name=f"zr{qc}_{s}")
                nc.vector.reciprocal(zrec, otv[:, :, D])
                osb = epi.tile([P, HL, D], BF16, tag="osb",
                               name=f"osb{qc}_{s}")
                zb = bass.AP(
                    tensor=zrec.tensor, offset=zrec.offset,
                    ap=[*zrec.ap, [0, D]])
                nc.vector.tensor_mul(osb, otv[:, :, 0:D], zb)
                osbs.append(osb)
            return osbs

        def epi_stile(qc, s, osb, last=False):
            q0 = qc * QCHUNK
            osf = osb.rearrange("p h d -> p (h d)")
            otp = o_pool.tile([P, CH_T, P], BF16, tag="ot",
                              name=f"otp{qc}_{s}")
            for ct in range(CH_T):
                nc.tensor.transpose(
                    otp[:, ct, :], osf[:, ct * P:(ct + 1) * P], idn)
            otsb = otsb_pool.tile([P, CH_T, P], BF16, tag="otsb",
                                  name=f"otsb{qc}_{s}")
            if last:
                nc.scalar.copy(otsb, otp)
            else:
                nc.vector.tensor_copy(otsb, otp)
            ppss = []
            for cf, (c0, c1) in enumerate(((0, CH), (CH, C))):
                pps = o_pool.tile([P, CH], F32, tag="ot",
                                  name=f"pps{qc}_{s}_{cf}")
                for ct in range(CH_T):
                    nc.tensor.matmul(
                        pps,
                        otsb[:, ct, :],
                        wp_sb[ct][:, c0:c1],
                        start=(ct == 0), stop=(ct == CH_T - 1))
                ppss.append(pps)
            ob = outsb_pool.tile([P, C], F32, tag="ob", name=f"ob{qc}_{s}")
            if last:
                nc.scalar.copy(ob[:, 0:CH], ppss[0])
                nc.vector.tensor_copy(ob[:, CH:C], ppss[1])
            else:
                nc.vector.tensor_copy(ob[:, 0:CH], ppss[0])
                nc.vector.tensor_copy(ob[:, CH:C], ppss[1])
            nc.sync.dma_start(
                out=out[q0 + s * P:q0 + (s + 1) * P, :], in_=ob)

        # ---- emission ----
        # prefix: Q/K projections for head pair 0, q/k block 0
        unit_q(0, 0, wq_sb, qt_sb)
        unit_q(0, 0, wk_sb, kt_sb)
        make_identity(nc, idn)

        def run_qchunk(qc, window, window_units, post_units, drain_rate):
            otiles = None
            wi = pi = 0
            drain_credit = 0.0
            npost = NIT - window - 6  # finish post units ~6 iters early
            for it in range(NIT):
                h, ktp = divmod(it, NKT // 2)
                if it == window:
                    otiles = [o_pool.tile([P, HL * E], F32, tag="ot",
                                          name=f"otile{qc}_{s_}")
                              for s_ in range(QSUBS)]
                    old = [(g[0], g[1], g[2]) for g in pvq]
                    pvq.clear()
                    for gh, gktp, gpm in old:
                        pvq.append((gh, gktp, gpm, otiles))
                emit_qk_exp_mask(qc, h, ktp, otiles)
                if it < window:
                    due = ((it + 1) * len(window_units) + window - 1) // window
                    while wi < min(due, len(window_units)):
                        window_units[wi]()
                        wi += 1
                else:
                    k = it - window + 1
                    due = (k * len(post_units) + npost - 1) // npost
                    while pi < min(due, len(post_units)):
                        post_units[pi]()
                        pi += 1
                    drain_credit = min(drain_credit + drain_rate, 4.0)
                    while (drain_credit >= 1.0 and len(pvq) > 2
                           and front_ready(qc)):
                        emit_pv_group()
                        drain_credit -= 1.0
            while pvq:
                emit_pv_group()
            return otiles

        pending = None
        for qc in range(NQC):
            if qc == 0:
                # 32 units: V(16), K(0,1..3), K(1,*), K(2,*), Q(1,0),
                # Q(2,0), Q(*,1).  Deadlines: K(0,n) before iter 2n;
                # K/Q(1,*) before iter 16; K/Q(2,*) before iter 32;
                # V(j) before the h0 drain; Q(*,1) before qc1.
                window = 26
                window_units = (
                    [lambda: unit_v(0),
                     lambda: unit_q(0, 1, wk_sb, kt_sb),
                     lambda: unit_v(1),
                     lambda: unit_q(0, 2, wk_sb, kt_sb),
                     lambda: unit_v(2),
                     lambda: unit_q(0, 3, wk_sb, kt_sb),
                     lambda: unit_v(3),
                     lambda: unit_v(4),
                     lambda: unit_q(1, 0, wq_sb, qt_sb)]
                    + [lambda n=n: unit_q(1, n, wk_sb, kt_sb)
                       for n in range(NQC)]
                    + [lambda j=j: unit_v(j) for j in (5, 6, 7, 8)]
                    + [lambda: unit_q(2, 0, wq_sb, qt_sb)]
                    + [lambda n=n: unit_q(2, n, wk_sb, kt_sb)
                       for n in range(NQC)]
                    + [lambda j=j: unit_v(j) for j in range(9, NKT)]
                    + [lambda m=m: unit_q(m, 1, wq_sb, qt_sb)
                       for m in range(MP)])
                drain_rate = 1.4
            else:
                window = 12
                pqc, potiles = pending
                osbs = epi_divisions(pqc, potiles)
                window_units = []
                for s in range(QSUBS):
                    window_units.append(
                        lambda s=s, o=osbs[s], p=pqc: epi_stile(p, s, o))
                    if qc < NQC - 1 and s < MP:
                        window_units.append(
                            lambda m=s, n=qc + 1: unit_q(m, n, wq_sb, qt_sb))
                drain_rate = 1.15
            otiles = run_qchunk(qc, window, window_units, [], drain_rate)
            if qc + 1 < NQC:
                issue_mask(qc + 1)
            pending = (qc, otiles)

        # final epilogue, eagerly pipelined per s-tile (ACT idle: use it)
        pqc, potiles = pending
        osbs = epi_divisions(pqc, potiles)
        for s in range(QSUBS):
            epi_stile(pqc, s, osbs[s], last=True)

    nc.compile()
    return nc


_CACHE = {}


def _get_nc():
    if "nc" not in _CACHE:
        _CACHE["nc"] = build_kernel()
    return _CACHE["nc"]


def kernel(x, mask, Wq, Wkv, Wp, bp):
    x = np.asarray(x, np.float32)
    mask = np.asarray(mask, np.float32)
    Wq = np.asarray(Wq, np.float32)
    Wkv = np.asarray(Wkv, np.float32)
    Wp = np.asarray(Wp, np.float32)
    bp = np.asarray(bp, np.float32)

    nc = _get_nc()
    in_maps = []
    for core in range(NCORES):
        b, hg = divmod(core, HGROUPS)
        rows = slice(hg * CH, (hg + 1) * CH)
        in_maps.append({
            "xT": np.ascontiguousarray(x[b].T.astype(ml_dtypes.bfloat16)),
            "wqt": np.ascontiguousarray(((Wq[rows, :] * SCALE).T).astype(ml_dtypes.bfloat16)),
            "wkt": np.ascontiguousarray(Wkv[rows, :].T.astype(ml_dtypes.bfloat16)),
            "wvt": np.ascontiguousarray(Wkv.T[:, C + hg * CH:C + (hg + 1) * CH].astype(ml_dtypes.bfloat16)),
            "wpt": np.ascontiguousarray(Wp[:, rows].T.astype(ml_dtypes.bfloat16)),
            "negmt": np.ascontiguousarray(
                (1.0 - mask[b].T).astype(ml_dtypes.bfloat16)),
        })

    trace = os.environ.get("KERNEL_TRACE", "0") == "1"
    if os.environ.get("KERNEL_WARMUP", "1") == "1":
        run_bass_kernel_spmd(nc, in_maps, core_ids=list(range(NCORES)),
                             trace=False)
    res = run_bass_kernel_spmd(nc, in_maps, core_ids=list(range(NCORES)),
                               trace=trace)
    kernel.last_results = res

    outs = [res.results[i]["out"] for i in range(NCORES)]
    full = np.empty((B, N, C), np.float32)
    for b in range(B):
        full[b] = outs[2 * b] + outs[2 * b + 1] + bp[None, :]
    return full


# revision 21
# speedup vs baseline: 1.2157x; 1.2157x over previous
"""Distributed Trainium2 Bass kernel for masked multi-head attention.

Problem: out = (softmax(scale * x Wq^T (x Wk^T)^T + mask * -1e5) (x Wv^T)) Wp^T + bp
  x [4, 2048, 768], mask [4, 2048, 2048], H=12 heads, D=64.

Sharding (8 cores): core = (batch b, head-group hg) with b = core//2,
hg = core%2 (6 heads each).  Column-parallel Wq/Wkv, row-parallel Wp;
each core produces a partial [2048, 768] output; the host sums the two
head-group partials per batch and adds the bias, then stacks batches.

Device schedule: the ACT engine's exp stream (192 tiles x ~1.03us) is
the pace setter; everything is arranged so it starts early and rarely
waits:
  - Minimal prefix (Q/K projection of head-pair 0 / block 0 on
    fine-grained x^T DMA) so the first QK->exp fires at ~8us instead of
    after a 60us serial projection phase.
  - All other projections drip into the attention iteration stream as
    PE filler.  Their PSUM comes from the shared "ot" slots, kept free
    by deferring early PV matmuls (pm tiles queue in a deep SBUF pool
    and drain after the per-qchunk O accumulators allocate); overflow
    units borrow "ring" slots once the otiles exist.
  - qchunk boundaries: previous epilogue (softmax divide / transpose /
    out-projection) and next q-block projections run in a deferred-PV
    window at the top of each qchunk; divisions are emitted first in
    DVE program order so the otile slots free immediately.
  - Engines: ACT = exp only; DVE = mask-mul + divisions + even-head
    evictions; Pool = odd-head evictions, psum->sbuf copies, mask DMA
    issue; SP = x/w/out DMA.
  - qt/kt pack head PAIRS on partitions (rows 0:64 even head, 64:128
    odd head): QK contracts over 64 rows directly, evictions stay on
    their own partitions, SBUF halves.
"""

import os
from collections import deque
from contextlib import ExitStack

import ml_dtypes
import numpy as np

import sys
import types

try:  # defensive: concourse's trace path imports this; absent on some images
    import antenv.axon_hooks  # noqa: F401
except ImportError:
    try:
        import antenv
        _m = types.ModuleType('antenv.axon_hooks')
        _m._hook = None
        _m.set_axon_ntff_profile_hook = lambda h: setattr(_m, '_hook', h)
        _m.get_axon_ntff_profile_hook = lambda: _m._hook
        sys.modules['antenv.axon_hooks'] = _m
        antenv.axon_hooks = _m
    except ImportError:
        pass

import concourse.bass as bass
import concourse.tile as tile
from concourse import bacc, mybir
from concourse.bass_utils import run_bass_kernel_spmd
from concourse.masks import make_identity

B, N, C, H, D = 4, 2048, 768, 12, 64
SCALE = D ** -0.5
NCORES = 8
HGROUPS = 2
HL = H // HGROUPS          # 6 heads per group
CH = HL * D                # 384 channels per group
P = 128
NKT = N // P               # 16 k tiles
QCHUNK = 512
NQC = N // QCHUNK          # 4 q chunks
QSUBS = QCHUNK // P        # 4
CIN_T = C // P             # 6 input-channel tiles
CH_T = CH // P             # 3 group-channel tiles
MP = HL // 2               # 3 head pairs per group
E = D + 1                  # head slot width in O psum (64 V cols + 1 ones col)
NIT = HL * NKT // 2        # 48 iterations per qchunk

F32 = mybir.dt.float32
BF16 = mybir.dt.bfloat16


def build_kernel():
    nc = bacc.Bacc("TRN2", target_bir_lowering=False, debug=False,
                   num_devices=NCORES)

    xT = nc.dram_tensor("xT", [C, N], BF16, kind="ExternalInput").ap()
    wqt = nc.dram_tensor("wqt", [C, CH], BF16, kind="ExternalInput").ap()
    wkt = nc.dram_tensor("wkt", [C, CH], BF16, kind="ExternalInput").ap()
    wvt = nc.dram_tensor("wvt", [C, CH], BF16, kind="ExternalInput").ap()
    wpt = nc.dram_tensor("wpt", [CH, C], BF16, kind="ExternalInput").ap()
    negmt = nc.dram_tensor("negmt", [N, N], BF16, kind="ExternalInput").ap()
    out = nc.dram_tensor("out", [N, C], F32, kind="ExternalOutput").ap()

    with tile.TileContext(nc) as tc, ExitStack() as ctx:
        persist = ctx.enter_context(tc.tile_pool(name="persist", bufs=1))
        ring_pool = ctx.enter_context(
            tc.tile_pool(name="ring", bufs=2, space="PSUM"))
        o_pool = ctx.enter_context(
            tc.tile_pool(name="opsum", bufs=4, space="PSUM"))

        # head-pair packed Q^T / K^T: rows 0:64 head 2m, rows 64:128 head 2m+1
        qt_sb = [persist.tile([P, N], BF16, tag=f"qt{m}", name=f"qt{m}")
                 for m in range(MP)]
        kt_sb = [persist.tile([P, N], BF16, tag=f"kt{m}", name=f"kt{m}")
                 for m in range(MP)]
        vp_sb = [persist.tile([P, HL, E], BF16, tag=f"vp{j}", name=f"vp{j}")
                 for j in range(NKT)]
        wp_sb = [persist.tile([P, C], BF16, tag=f"wp{t}", name=f"wp{t}")
                 for t in range(CH_T)]
        idn = persist.tile([P, P], BF16, tag="idn")

        ph1 = ctx.enter_context(tc.tile_pool(name="ph1", bufs=1))
        xt_sb = [ph1.tile([P, N], BF16, tag=f"xt{i}", name=f"xt{i}")
                 for i in range(CIN_T)]
        wq_sb = [ph1.tile([P, CH], BF16, tag=f"wq{i}", name=f"wq{i}")
                 for i in range(CIN_T)]
        wk_sb = [ph1.tile([P, CH], BF16, tag=f"wk{i}", name=f"wk{i}")
                 for i in range(CIN_T)]
        wv_sb = [ph1.tile([P, CH], BF16, tag=f"wv{i}", name=f"wv{i}")
                 for i in range(CIN_T)]

        # ---- DMA issue (SP queue): only the bytes the prefix needs first
        # (wq/wk head-pair-0 columns + x^T q-block 0), then the rest.
        for i in range(CIN_T):
            sl = slice(i * P, (i + 1) * P)
            nc.sync.dma_start(out=wq_sb[i][:, 0:P], in_=wqt[sl, 0:P])
            nc.sync.dma_start(out=wk_sb[i][:, 0:P], in_=wkt[sl, 0:P])
        for i in range(CIN_T):
            nc.sync.dma_start(out=xt_sb[i][:, 0:QCHUNK],
                              in_=xT[i * P:(i + 1) * P, 0:QCHUNK])
        for i in range(CIN_T):
            sl = slice(i * P, (i + 1) * P)
            nc.sync.dma_start(out=wv_sb[i], in_=wvt[sl, :])
        for i in range(CIN_T):
            sl = slice(i * P, (i + 1) * P)
            nc.sync.dma_start(out=wq_sb[i][:, P:CH], in_=wqt[sl, P:CH])
            nc.sync.dma_start(out=wk_sb[i][:, P:CH], in_=wkt[sl, P:CH])
        for nck in range(1, NQC):
            q0 = nck * QCHUNK
            for i in range(CIN_T):
                nc.sync.dma_start(out=xt_sb[i][:, q0:q0 + QCHUNK],
                                  in_=xT[i * P:(i + 1) * P, q0:q0 + QCHUNK])
        for t in range(CH_T):
            nc.sync.dma_start(out=wp_sb[t], in_=wpt[t * P:(t + 1) * P, :])

        # ---- mask tiles (Pool queue; 2 rotating slots) ----
        mpool = ctx.enter_context(tc.tile_pool(name="mask", bufs=2))
        mk_tiles = {}

        def issue_mask(qc):
            q0 = qc * QCHUNK
            mk = mpool.tile([P, NKT, QCHUNK], BF16, tag="mk", name=f"mk{qc}")
            for j in range(NKT):
                nc.gpsimd.dma_start(
                    out=mk[:, j, :],
                    in_=negmt[j * P:(j + 1) * P, q0:q0 + QCHUNK])
            mk_tiles[qc] = mk

        issue_mask(0)
        issue_mask(1)

        # ---- pointwise pools ----
        p_pool = ctx.enter_context(tc.tile_pool(name="pexp", bufs=3))
        pm_pool = ctx.enter_context(tc.tile_pool(name="pmask", bufs=12))
        epi = ctx.enter_context(tc.tile_pool(name="epi", bufs=6))
        otsb_pool = ctx.enter_context(tc.tile_pool(name="otsb", bufs=4))
        outsb_pool = ctx.enter_context(tc.tile_pool(name="outsb", bufs=2))

        # ---- projection units (one psum slot + eviction each) ----
        v_emitted = set()

        def unit_q(m, n, w_sb, dst):
            ps = o_pool.tile([P, QCHUNK], F32, tag="ot", name=f"pj{m}_{n}")
            for ci in range(CIN_T):
                nc.tensor.matmul(
                    ps,
                    w_sb[ci][:, m * P:(m + 1) * P],
                    xt_sb[ci][:, n * QCHUNK:(n + 1) * QCHUNK],
                    start=(ci == 0), stop=(ci == CIN_T - 1))
            dt = dst[m]
            nsl = slice(n * QCHUNK, (n + 1) * QCHUNK)
            # both packed heads in one copy (GPSIMD cannot read PSUM)
            nc.vector.tensor_copy(dt[:, nsl], ps)

        def unit_v(j):
            ps = o_pool.tile([P, CH], F32, tag="ot", name=f"vps{j}")
            for ci in range(CIN_T):
                nc.tensor.matmul(
                    ps,
                    xt_sb[ci][:, j * P:(j + 1) * P],
                    wv_sb[ci],
                    start=(ci == 0), stop=(ci == CIN_T - 1))
            nc.gpsimd.memset(vp_sb[j], 1.0)
            nc.vector.tensor_copy(
                vp_sb[j][:, :, 0:D],
                ps.rearrange("p (h d) -> p h d", h=HL))
            v_emitted.add(j)

        # ---- attention iteration pieces ----
        # iterations come in pairs: two QK+exp (2 k-tiles each) fill one
        # [P, 4, QCHUNK] pexp tile; one mask-mul + one 4-k-tile PV group
        # per pair (halves the DVE instruction count).
        pvq = deque()          # deferred PV groups: (h, g, pm, otiles)
        cur_pexp = [None]

        def emit_qk_exp(qc, h, ktp, otiles):
            m, r = divmod(h, 2)
            rows = slice(r * D, (r + 1) * D)
            q0 = qc * QCHUNK
            half = ktp % 2
            ring = ring_pool.tile([P, 2, QCHUNK], F32, tag="ring",
                                  name=f"ring{qc}_{h}_{ktp}")
            for u in range(2):
                kti = 2 * ktp + u
                nc.tensor.matmul(
                    ring[:, u, :],
                    kt_sb[m][rows, kti * P:(kti + 1) * P],
                    qt_sb[m][rows, q0:q0 + QCHUNK],
                    start=True, stop=True)
            if half == 0:
                cur_pexp[0] = p_pool.tile([P, 4, QCHUNK], BF16, tag="pe",
                                          name=f"pe{qc}_{h}_{ktp}")
            pexp = cur_pexp[0]
            nc.scalar.activation(
                pexp[:, 2 * half:2 * half + 2, :], ring,
                mybir.ActivationFunctionType.Exp)
            if half == 1:
                g = ktp // 2           # 4-k-tile group within the head
                pm = pm_pool.tile([P, 4, QCHUNK], BF16, tag="pm",
                                  name=f"pm{qc}_{h}_{g}")
                nc.vector.tensor_mul(
                    pm, pexp, mk_tiles[qc][:, 4 * g:4 * g + 4, :])
                pvq.append((h, g, pm, otiles))

        def front_ready(qc):
            if qc > 0:
                return True
            g = pvq[0][1]
            return all(4 * g + u in v_emitted for u in range(4))

        def emit_pv_group():
            h, g, pm, otiles = pvq.popleft()
            for u in range(4):
                kti = 4 * g + u
                for s in range(QSUBS):
                    nc.tensor.matmul(
                        otiles[s][:, h * E:(h + 1) * E],
                        pm[:, u, s * P:(s + 1) * P],
                        vp_sb[kti][:, h, :],
                        start=(kti == 0), stop=(kti == NKT - 1))

        # ---- epilogue: divisions first, then per-s-tile chunks ----
        def epi_divisions(qc, otiles):
            osbs = []
            for s in range(QSUBS):
                otv = otiles[s].rearrange("p (h e) -> p h e", h=HL)
                zrec = epi.tile([P, HL], F32, tag="zr", name=f"zr{qc}_{s}")
                nc.vector.reciprocal(zrec, otv[:, :, D])
                osb = epi.tile([P, HL, D], BF16, tag="osb",
                               name=f"osb{qc}_{s}")
                zb = bass.AP(
                    tensor=zrec.tensor, offset=zrec.offset,
                    ap=[*zrec.ap, [0, D]])
                nc.vector.tensor_mul(osb, otv[:, :, 0:D], zb)
                osbs.append(osb)
            return osbs

        def epi_stile(qc, s, osb, last=False):
            q0 = qc * QCHUNK
            osf = osb.rearrange("p h d -> p (h d)")
            otp = o_pool.tile([P, CH_T, P], BF16, tag="ot",
                              name=f"otp{qc}_{s}")
            for ct in range(CH_T):
                nc.tensor.transpose(
                    otp[:, ct, :], osf[:, ct * P:(ct + 1) * P], idn)
            otsb = otsb_pool.tile([P, CH_T, P], BF16, tag="otsb",
                                  name=f"otsb{qc}_{s}")
            # ACT: lands in the PE-bound boundary window where exp idles
            nc.scalar.copy(otsb, otp)
            ppss = []
            for cf, (c0, c1) in enumerate(((0, CH), (CH, C))):
                pps = o_pool.tile([P, CH], F32, tag="ot",
                                  name=f"pps{qc}_{s}_{cf}")
                for ct in range(CH_T):
                    nc.tensor.matmul(
                        pps,
                        otsb[:, ct, :],
                        wp_sb[ct][:, c0:c1],
                        start=(ct == 0), stop=(ct == CH_T - 1))
                ppss.append(pps)
            ob = outsb_pool.tile([P, C], F32, tag="ob", name=f"ob{qc}_{s}")
            if last:
                nc.scalar.copy(ob[:, 0:CH], ppss[0])
                nc.vector.tensor_copy(ob[:, CH:C], ppss[1])
            else:
                nc.vector.tensor_copy(ob[:, 0:CH], ppss[0])
                nc.vector.tensor_copy(ob[:, CH:C], ppss[1])
            nc.sync.dma_start(
                out=out[q0 + s * P:q0 + (s + 1) * P, :], in_=ob)

        # ---- emission ----
        # prefix: Q/K projections for head pair 0, q/k block 0
        unit_q(0, 0, wq_sb, qt_sb)
        unit_q(0, 0, wk_sb, kt_sb)
        make_identity(nc, idn)

        def run_qchunk(qc, window, window_units, drain_rate):
            otiles = None
            wi = 0
            drain_credit = 0.0
            for it in range(NIT):
                h, ktp = divmod(it, NKT // 2)
                if it == window:
                    otiles = [o_pool.tile([P, HL * E], F32, tag="ot",
                                          name=f"otile{qc}_{s_}")
                              for s_ in range(QSUBS)]
                    old = [(g[0], g[1], g[2]) for g in pvq]
                    pvq.clear()
                    for gh, gg, gpm in old:
                        pvq.append((gh, gg, gpm, otiles))
                emit_qk_exp(qc, h, ktp, otiles)
                if it < window:
                    due = ((it + 1) * len(window_units) + window - 1) // window
                    while wi < min(due, len(window_units)):
                        window_units[wi]()
                        wi += 1
                else:
                    drain_credit = min(drain_credit + drain_rate, 3.0)
                    while (drain_credit >= 1.0 and len(pvq) > 1
                           and front_ready(qc)):
                        emit_pv_group()
                        drain_credit -= 1.0
            while pvq:
                emit_pv_group()
            return otiles

        pending = None
        for qc in range(NQC):
            if qc == 0:
                # 32 units: V(16), K(0,1..3), K(1,*), K(2,*), Q(1,0),
                # Q(2,0), Q(*,1).  Deadlines: K(0,n) before iter 2n;
                # K/Q(1,*) before iter 16; K/Q(2,*) before iter 32;
                # V(j) before the h0 drain; Q(*,1) before qc1.
                window = 24
                window_units = (
                    [lambda: unit_v(0),
                     lambda: unit_q(0, 1, wk_sb, kt_sb),
                     lambda: unit_v(1),
                     lambda: unit_q(0, 2, wk_sb, kt_sb),
                     lambda: unit_v(2),
                     lambda: unit_q(0, 3, wk_sb, kt_sb),
                     lambda: unit_v(3),
                     lambda: unit_v(4),
                     lambda: unit_q(1, 0, wq_sb, qt_sb)]
                    + [lambda n=n: unit_q(1, n, wk_sb, kt_sb)
                       for n in range(NQC)]
                    + [lambda j=j: unit_v(j) for j in (5, 6, 7, 8)]
                    + [lambda: unit_q(2, 0, wq_sb, qt_sb)]
                    + [lambda n=n: unit_q(2, n, wk_sb, kt_sb)
                       for n in range(NQC)]
                    + [lambda j=j: unit_v(j) for j in range(9, NKT)]
                    + [lambda m=m: unit_q(m, 1, wq_sb, qt_sb)
                       for m in range(MP)])
                drain_rate = 1.0
            else:
                window = 12
                if qc + 1 < NQC:
                    issue_mask(qc + 1)
                pqc, potiles = pending
                osbs = epi_divisions(pqc, potiles)
                window_units = []
                for s in range(QSUBS):
                    window_units.append(
                        lambda s=s, o=osbs[s], p=pqc: epi_stile(p, s, o))
                    if qc < NQC - 1 and s < MP:
                        window_units.append(
                            lambda m=s, n=qc + 1: unit_q(m, n, wq_sb, qt_sb))
                drain_rate = 0.62
            otiles = run_qchunk(qc, window, window_units, drain_rate)
            pending = (qc, otiles)

        # ---- final epilogue: stage-parallel across the four s-tiles so
        # the post-exp tail is short; ACT and DVE split the copies ----
        pqc, potiles = pending
        osbs = epi_divisions(pqc, potiles)
        otps, otsbs, ppss_all, obs = [], [], [], []
        for s in range(QSUBS):
            osf = osbs[s].rearrange("p h d -> p (h d)")
            otp = o_pool.tile([P, CH_T, P], BF16, tag="ot", name=f"fotp{s}")
            for ct in range(CH_T):
                nc.tensor.transpose(
                    otp[:, ct, :], osf[:, ct * P:(ct + 1) * P], idn)
            otps.append(otp)
            otsb = otsb_pool.tile([P, CH_T, P], BF16, tag="otsb",
                                  name=f"fotsb{s}")
            if s % 2 == 0:
                nc.scalar.copy(otsb, otp)
            else:
                nc.vector.tensor_copy(otsb, otp)
            otsbs.append(otsb)
        for s in range(QSUBS):
            ppss = []
            for cf, (c0, c1) in enumerate(((0, CH), (CH, C))):
                pps = o_pool.tile([P, CH], F32, tag="ot", name=f"fpps{s}_{cf}")
                for ct in range(CH_T):
                    nc.tensor.matmul(
                        pps, otsbs[s][:, ct, :], wp_sb[ct][:, c0:c1],
                        start=(ct == 0), stop=(ct == CH_T - 1))
                ppss.append(pps)
            ob = outsb_pool.tile([P, C], F32, tag="ob", name=f"fob{s}")
            nc.scalar.copy(ob[:, 0:CH], ppss[0])
            nc.vector.tensor_copy(ob[:, CH:C], ppss[1])
            nc.sync.dma_start(
                out=out[pqc * QCHUNK + s * P:pqc * QCHUNK + (s + 1) * P, :],
                in_=ob)

    nc.compile()
    return nc


_CACHE = {}


def _get_nc():
    if "nc" not in _CACHE:
        _CACHE["nc"] = build_kernel()
    return _CACHE["nc"]


def kernel(x, mask, Wq, Wkv, Wp, bp):
    x = np.asarray(x, np.float32)
    mask = np.asarray(mask, np.float32)
    Wq = np.asarray(Wq, np.float32)
    Wkv = np.asarray(Wkv, np.float32)
    Wp = np.asarray(Wp, np.float32)
    bp = np.asarray(bp, np.float32)

    nc = _get_nc()
    in_maps = []
    for core in range(NCORES):
        b, hg = divmod(core, HGROUPS)
        rows = slice(hg * CH, (hg + 1) * CH)
        in_maps.append({
            "xT": np.ascontiguousarray(x[b].T.astype(ml_dtypes.bfloat16)),
            "wqt": np.ascontiguousarray(((Wq[rows, :] * SCALE).T).astype(ml_dtypes.bfloat16)),
            "wkt": np.ascontiguousarray(Wkv[rows, :].T.astype(ml_dtypes.bfloat16)),
            "wvt": np.ascontiguousarray(Wkv.T[:, C + hg * CH:C + (hg + 1) * CH].astype(ml_dtypes.bfloat16)),
            "wpt": np.ascontiguousarray(Wp[:, rows].T.astype(ml_dtypes.bfloat16)),
            "negmt": np.ascontiguousarray(
                (1.0 - mask[b].T).astype(ml_dtypes.bfloat16)),
        })

    trace = os.environ.get("KERNEL_TRACE", "0") == "1"
    if os.environ.get("KERNEL_WARMUP", "1") == "1":
        run_bass_kernel_spmd(nc, in_maps, core_ids=list(range(NCORES)),
                             trace=False)
    res = run_bass_kernel_spmd(nc, in_maps, core_ids=list(range(NCORES)),
                               trace=trace)
    kernel.last_results = res

    outs = [res.results[i]["out"] for i in range(NCORES)]
    full = np.empty((B, N, C), np.float32)
    for b in range(B):
        full[b] = outs[2 * b] + outs[2 * b + 1] + bp[None, :]
    return full
